# revision 1
# baseline (speedup 1.0000x reference)
"""Distributed GCN (2-layer DGL GraphConv) on 8 Trainium2 NeuronCores.

Strategy
--------
- Nodes sorted by in-degree and packed into 128-node tiles; tiles striped
  across the 8 cores (tile t -> core t%8, position t//8) so that each
  position holds similar-degree nodes on every core (SPMD-identical shapes).
- Layer projection G1' = norm_src * (x @ W1) computed on the owning core in
  bf16 (W1 and x split into hi+lo bf16 pairs for fp32-like accuracy), fp32
  PSUM accumulate, stored to an fp16 table block and AllGathered so every
  core holds the full [8*B, 256] node-feature table.
- Message passing = dma_gather of each dst-node's in-neighbor rows laid out
  as [dst-partition, slot] + a single strided VectorE reduce over slots.
  Graph norms are folded into per-partition activation scales.
- Layer 2 aggregates scalars g2 = relu-layer-1 . W2 via an identical gather
  from a broadcast fp32 table using the same index arrays.
- int16 gather indices cover only 32768 rows, so each node's slots are split
  into a "lo" group (table rows < 5*B) and a rebased "hi" group; pad slots
  point at per-block zero rows.
"""
import os
import sys

for _p in ("/opt/trn_rl_repo", "/root/.axon_site/_ro/trn_rl_repo"):
    if os.path.isdir(_p) and _p not in sys.path:
        sys.path.insert(0, _p)

import numpy as np
import ml_dtypes

C = 8          # cores
P = 128        # partitions
B = 6400       # table block rows per core
D = 1024
DH = 256
KT = D // P

bf16 = ml_dtypes.bfloat16


# --------------------------------------------------------------------------
# host-side preprocessing
# --------------------------------------------------------------------------
def _preprocess(x, src, dst, W1, b1, W2, b2, n_nodes):
    N = int(n_nodes)
    E = src.shape[0]
    src = np.asarray(src).astype(np.int64)
    dst = np.asarray(dst).astype(np.int64)
    x = np.asarray(x, dtype=np.float32)
    W1 = np.asarray(W1, dtype=np.float32)
    b1 = np.asarray(b1, dtype=np.float32).reshape(-1)
    W2 = np.asarray(W2, dtype=np.float32)
    b2 = np.asarray(b2, dtype=np.float32).reshape(-1)

    n_tiles = (N + P - 1) // P
    TPC = (n_tiles + C - 1) // C
    NPAD = TPC * P
    assert NPAD + 1 <= B, "node count exceeds table block capacity"

    deg_out = np.bincount(src, minlength=N).astype(np.float32)
    deg_in = np.bincount(dst, minlength=N).astype(np.float32)
    norm_src = np.clip(deg_out, 1.0, None) ** -0.5
    norm_dst = np.clip(deg_in, 1.0, None) ** -0.5

    order = np.argsort(-deg_in, kind="stable")
    core_of = np.empty(N, np.int32)
    pos_of = np.empty(N, np.int32)
    part_of = np.empty(N, np.int32)
    for t in range(n_tiles):
        nodes = order[t * P:(t + 1) * P]
        core_of[nodes] = t % C
        pos_of[nodes] = t // C
        part_of[nodes] = np.arange(len(nodes))
    row_of = core_of.astype(np.int64) * B + 1 + pos_of.astype(np.int64) * P + part_of
    loc_of = pos_of * P + part_of

    # lo/hi split for the int16 index limit
    SPLIT_CORES = min(C - 1, 32767 // B)
    LO_ROWS = SPLIT_CORES * B
    e_is_hi = (row_of[src] >= LO_ROWS)

    deg_lo = np.bincount(dst[~e_is_hi], minlength=N).astype(np.int64)
    deg_hi = np.bincount(dst[e_is_hi], minlength=N).astype(np.int64)
    S_lo = np.zeros(TPC, np.int64)
    S_hi = np.zeros(TPC, np.int64)
    for j in range(TPC):
        sel = pos_of == j
        S_lo[j] = max(int(deg_lo[sel].max()), 1) if sel.any() else 1
        S_hi[j] = max(int(deg_hi[sel].max()), 1) if sel.any() else 1
    cols_lo = (S_lo * P // 16).astype(np.int64)
    cols_hi = (S_hi * P // 16).astype(np.int64)
    seq = np.empty(2 * TPC, np.int64)
    seq[0::2] = cols_lo
    seq[1::2] = cols_hi
    off = np.r_[0, np.cumsum(seq)][:-1]
    col_off_lo = off[0::2]
    col_off_hi = off[1::2]
    TOTC = int(seq.sum())

    e_order = np.lexsort((row_of[src], e_is_hi, row_of[dst]))
    ds = dst[e_order]
    ss = src[e_order]
    hs = e_is_hi[e_order]
    key = row_of[ds] * 2 + hs
    grp_start = np.r_[True, key[1:] != key[:-1]]
    starts = np.flatnonzero(grp_start)
    idx_in_grp = np.arange(E) - np.repeat(starts, np.diff(np.r_[starts, E]))
    e_core = core_of[ds]
    i_within = idx_in_grp * P + part_of[ds]
    base_col = np.where(hs, col_off_hi[pos_of[ds]], col_off_lo[pos_of[ds]])
    i_col = base_col + i_within // 16
    i_row = i_within % 16
    vals = np.where(hs, row_of[ss] - LO_ROWS, row_of[ss])
    flat = np.zeros((C, 16 * TOTC), np.uint16)  # pad -> idx 0 (zero row, both halves)
    flat[e_core, i_row * TOTC + i_col] = vals.astype(np.uint16)
    idx16 = np.tile(flat.reshape(C, 16, TOTC), (1, 8, 1)).view(np.int16)

    x_hi = x.astype(bf16)
    x_lo = (x - x_hi.astype(np.float32)).astype(bf16)
    xT_hi = np.zeros((C, TPC, P, KT * P), bf16)
    xT_lo = np.zeros((C, TPC, P, KT * P), bf16)
    ns = np.zeros((C, P, TPC), np.float32)
    scomb = np.zeros((C, P, TPC), np.float32)
    nd = np.zeros((C, P, TPC), np.float32)
    for c in range(C):
        sel = np.flatnonzero(core_of == c)
        loc = loc_of[sel]
        xh = np.zeros((NPAD, D), bf16)
        xl = np.zeros((NPAD, D), bf16)
        xh[loc] = x_hi[sel]
        xl[loc] = x_lo[sel]
        # param[j, p, k*P+m] = x[node j*P+m, feat k*P+p] -> one 256 KB DMA/position
        xT_hi[c] = xh.reshape(TPC, P, KT, P).transpose(0, 3, 2, 1).reshape(TPC, P, KT * P)
        xT_lo[c] = xl.reshape(TPC, P, KT, P).transpose(0, 3, 2, 1).reshape(TPC, P, KT * P)
        ns[c, part_of[sel], pos_of[sel]] = norm_src[sel]
        scomb[c, part_of[sel], pos_of[sel]] = norm_src[sel] * norm_dst[sel]
        nd[c, part_of[sel], pos_of[sel]] = norm_dst[sel]

    W_hi = W1.astype(bf16)
    W_lo = (W1 - W_hi.astype(np.float32)).astype(bf16)

    return dict(
        N=N, TPC=TPC, NPAD=NPAD, B=B, LO_ROWS=LO_ROWS,
        S_lo=S_lo, S_hi=S_hi, cols_lo=cols_lo, cols_hi=cols_hi,
        col_off_lo=col_off_lo, col_off_hi=col_off_hi, TOTC=TOTC,
        idx16=idx16, xT_hi=xT_hi, xT_lo=xT_lo,
        Whi=W_hi.reshape(KT, P, DH), Wlo=W_lo.reshape(KT, P, DH),
        ns=ns, scomb=scomb, nd=nd,
        w2rep=np.tile(W2.reshape(1, -1), (P, 1)).astype(np.float32),
        b1rep=np.tile(b1.reshape(1, -1), (P, 1)).astype(np.float32),
        b1_any=bool(np.any(b1 != 0)), b2=float(b2[0]),
        core_of=core_of, pos_of=pos_of, part_of=part_of,
    )


# --------------------------------------------------------------------------
# device program
# --------------------------------------------------------------------------
def _build(pp, xsplit=True):
    import concourse.bacc as bacc
    import concourse.mybir as mybir
    import concourse.tile as tile
    from concourse import library_config

    TPC_used = pp["TPC"]
    S_lo, S_hi = pp["S_lo"], pp["S_hi"]
    col_off_lo, col_off_hi = pp["col_off_lo"], pp["col_off_hi"]
    cols_lo, cols_hi = pp["cols_lo"], pp["cols_hi"]
    TOTC = pp["TOTC"]
    LO_ROWS = pp["LO_ROWS"]
    b1_any = pp["b1_any"]
    Bv = pp["B"]
    NPAD = TPC_used * P
    TBL = C * Bv
    f32, f16, bf16_my, i16 = (mybir.dt.float32, mybir.dt.float16,
                              mybir.dt.bfloat16, mybir.dt.int16)

    nc = bacc.Bacc("TRN2", target_bir_lowering=False, debug=False, num_devices=C)

    xT_hi = nc.declare_dram_parameter("xT_hi", [TPC_used, P, KT * P], bf16_my, isOutput=False)
    if xsplit:
        xT_lo = nc.declare_dram_parameter("xT_lo", [TPC_used, P, KT * P], bf16_my, isOutput=False)
    Whi = nc.declare_dram_parameter("Whi", [KT, P, DH], bf16_my, isOutput=False)
    Wlo = nc.declare_dram_parameter("Wlo", [KT, P, DH], bf16_my, isOutput=False)
    idxp = nc.declare_dram_parameter("idx", [P, TOTC], i16, isOutput=False)
    nsp = nc.declare_dram_parameter("ns", [P, TPC_used], f32, isOutput=False)
    scombp = nc.declare_dram_parameter("scomb", [P, TPC_used], f32, isOutput=False)
    ndp = nc.declare_dram_parameter("nd", [P, TPC_used], f32, isOutput=False)
    w2p = nc.declare_dram_parameter("w2rep", [P, DH], f32, isOutput=False)
    if b1_any:
        b1p = nc.declare_dram_parameter("b1rep", [P, DH], f32, isOutput=False)
    b2_any = (float(pp["b2"]) != 0.0)
    if b2_any:
        b2p = nc.declare_dram_parameter("b2rep", [P, 1], f32, isOutput=False)
    outp = nc.declare_dram_parameter("out", [NPAD, 1], f32, isOutput=True)

    tbl1c = nc.dram_tensor("tbl1c", [Bv, DH], f16)
    tbl1 = nc.dram_tensor("tbl1", [TBL, DH], f16, addr_space="Shared")
    tbl2c = nc.dram_tensor("tbl2c", [Bv, 64], f32)
    tbl2 = nc.dram_tensor("tbl2", [TBL, 64], f32, addr_space="Shared")

    groups = [list(range(C))]
    nc.gpsimd.load_library(library_config.mlp)

    with tile.TileContext(nc) as tc:
        with (
            tc.tile_pool(name="const", bufs=1) as constp,
            tc.tile_pool(name="xt", bufs=4) as xtp,
            tc.tile_pool(name="ps", bufs=4, space="PSUM") as psp,
            tc.tile_pool(name="g1o", bufs=3) as g1op,
            tc.tile_pool(name="gb", bufs=3) as gbp,
            tc.tile_pool(name="mid", bufs=3) as midp,
            tc.tile_pool(name="g2b", bufs=3) as g2bp,
        ):
            w_hi_sb = constp.tile([P, KT * DH], bf16_my, tag="whi")
            w_lo_sb = constp.tile([P, KT * DH], bf16_my, tag="wlo")
            for k in range(KT):
                nc.sync.dma_start(out=w_hi_sb[:, k * DH:(k + 1) * DH], in_=Whi[k])
                nc.sync.dma_start(out=w_lo_sb[:, k * DH:(k + 1) * DH], in_=Wlo[k])
            idx_sb = constp.tile([P, TOTC], i16, tag="idx")
            nc.sync.dma_start(out=idx_sb[:], in_=idxp[:])
            ns_sb = constp.tile([P, TPC_used], f32, tag="ns")
            nc.sync.dma_start(out=ns_sb[:], in_=nsp[:])
            scomb_sb = constp.tile([P, TPC_used], f32, tag="scomb")
            nc.sync.dma_start(out=scomb_sb[:], in_=scombp[:])
            nd_sb = constp.tile([P, TPC_used], f32, tag="nd")
            nc.sync.dma_start(out=nd_sb[:], in_=ndp[:])
            w2_sb = constp.tile([P, DH], f32, tag="w2")
            nc.sync.dma_start(out=w2_sb[:], in_=w2p[:])
            if b1_any:
                b1_sb = constp.tile([P, DH], f32, tag="b1")
                nc.sync.dma_start(out=b1_sb[:], in_=b1p[:])
            if b2_any:
                b2_sb = constp.tile([P, 1], f32, tag="b2")
                nc.sync.dma_start(out=b2_sb[:], in_=b2p[:])

            z16 = constp.tile([P, DH], f16, tag="z16")
            nc.vector.memset(z16[:], 0)
            z32 = constp.tile([P, 64], f32, tag="z32")
            nc.vector.memset(z32[:], 0)

            r = 1 + NPAD
            while r < Bv:
                n = min(P, Bv - r)
                nc.sync.dma_start(out=tbl1c[r:r + n, :], in_=z16[:n, :])
                nc.sync.dma_start(out=tbl2c[r:r + n, :], in_=z32[:n, :])
                r += n
            nc.sync.dma_start(out=tbl1c[0:1, :], in_=z16[:1, :])
            nc.sync.dma_start(out=tbl2c[0:1, :], in_=z32[:1, :])

            for j in range(TPC_used):
                ps = psp.tile([P, DH], f32, tag="ps")
                xh = xtp.tile([P, KT * P], bf16_my, tag="xh")
                nc.sync.dma_start(out=xh[:], in_=xT_hi[j])
                if xsplit:
                    xl = xtp.tile([P, KT * P], bf16_my, tag="xl")
                    nc.sync.dma_start(out=xl[:], in_=xT_lo[j])
                for k in range(KT):
                    nc.tensor.matmul(out=ps[:], lhsT=xh[:, k * P:(k + 1) * P],
                                     rhs=w_hi_sb[:, k * DH:(k + 1) * DH],
                                     start=(k == 0), stop=False)
                    last = (k == KT - 1)
                    nc.tensor.matmul(out=ps[:], lhsT=xh[:, k * P:(k + 1) * P],
                                     rhs=w_lo_sb[:, k * DH:(k + 1) * DH],
                                     start=False, stop=(last and not xsplit))
                    if xsplit:
                        nc.tensor.matmul(out=ps[:], lhsT=xl[:, k * P:(k + 1) * P],
                                         rhs=w_hi_sb[:, k * DH:(k + 1) * DH],
                                         start=False, stop=last)
                g1o = g1op.tile([P, DH], f16, tag="g1o")
                nc.scalar.activation(out=g1o[:], in_=ps[:],
                                     func=mybir.ActivationFunctionType.Copy,
                                     scale=ns_sb[:, j:j + 1])
                nc.sync.dma_start(out=tbl1c[1 + j * P:1 + (j + 1) * P, :], in_=g1o[:])

            nc.gpsimd.collective_compute(
                "AllGather", mybir.AluOpType.bypass, replica_groups=groups,
                ins=[tbl1c[:]], outs=[tbl1[:]],
            )

            SCAP = 24  # max slots per gather round (bounds SBUF for hub graphs)

            def gather_reduce(pool, j, table, fdim, dt, out_ap, in_view):
                """Gather all (lo+hi) slots of position j in SCAP-bounded
                rounds and reduce-accumulate into out_ap ([P, free])."""
                Sl, Sh = int(S_lo[j]), int(S_hi[j])
                work = []  # (slot0, nslots, base_col, table_slice)
                for s0 in range(0, Sl, SCAP):
                    work.append((s0, min(SCAP, Sl - s0), int(col_off_lo[j]),
                                 table[0:LO_ROWS, :]))
                for s0 in range(0, Sh, SCAP):
                    work.append((s0, min(SCAP, Sh - s0), int(col_off_hi[j]),
                                 table[LO_ROWS:, :]))
                first = True
                for s0, sc, bcol, tbl_ap in work:
                    gb = pool.tile([P, SCAP * fdim], dt, tag="gb")
                    gb3 = gb[:].rearrange("p (s f) -> p s f", f=fdim)
                    nc.gpsimd.dma_gather(
                        out_ap=gb3[:, 0:sc, :],
                        in_ap=tbl_ap,
                        idxs_ap=idx_sb[:, bcol + s0 * 8:bcol + (s0 + sc) * 8],
                        num_idxs=sc * P, num_idxs_reg=sc * P,
                        elem_size=fdim, single_packet=False,
                    )
                    red_in = in_view(gb, sc)
                    if first:
                        nc.vector.tensor_reduce(
                            out=out_ap, in_=red_in,
                            axis=mybir.AxisListType.X, op=mybir.AluOpType.add)
                        first = False
                    else:
                        tmp = pool.tile([P, out_ap.shape[1]], f32, tag="grtmp")
                        nc.vector.tensor_reduce(
                            out=tmp[:], in_=red_in,
                            axis=mybir.AxisListType.X, op=mybir.AluOpType.add)
                        nc.vector.tensor_tensor(
                            out=out_ap, in0=out_ap, in1=tmp[:],
                            op=mybir.AluOpType.add)

            for j in range(TPC_used):
                agg = midp.tile([P, DH], f32, tag="agg")
                gather_reduce(
                    gbp, j, tbl1, DH, f16, agg[:],
                    lambda gb, sc: gb[:, 0:sc * DH].rearrange(
                        "p (s f) -> p f s", f=DH))
                hp = midp.tile([P, DH], f32, tag="hp")
                if b1_any:
                    t1 = midp.tile([P, DH], f32, tag="t1")
                    nc.vector.tensor_tensor(
                        out=t1[:], in0=agg[:],
                        in1=nd_sb[:, j:j + 1].to_broadcast([P, DH]),
                        op=mybir.AluOpType.mult)
                    nc.vector.tensor_tensor(
                        out=t1[:], in0=t1[:], in1=b1_sb[:],
                        op=mybir.AluOpType.add)
                    nc.scalar.activation(out=hp[:], in_=t1[:],
                                         func=mybir.ActivationFunctionType.Relu,
                                         scale=ns_sb[:, j:j + 1])
                else:
                    nc.scalar.activation(out=hp[:], in_=agg[:],
                                         func=mybir.ActivationFunctionType.Relu,
                                         scale=scomb_sb[:, j:j + 1])
                prod = midp.tile([P, DH], f32, tag="prod")
                nc.vector.tensor_tensor(out=prod[:], in0=hp[:], in1=w2_sb[:],
                                        op=mybir.AluOpType.mult)
                g2c = midp.tile([P, 1], f32, tag="g2c")
                nc.vector.tensor_reduce(out=g2c[:], in_=prod[:],
                                        axis=mybir.AxisListType.X,
                                        op=mybir.AluOpType.add)
                g2b = midp.tile([P, 64], f32, tag="g2bc")
                nc.vector.tensor_copy(out=g2b[:], in_=g2c[:].to_broadcast([P, 64]))
                nc.sync.dma_start(out=tbl2c[1 + j * P:1 + (j + 1) * P, :], in_=g2b[:])

            nc.gpsimd.collective_compute(
                "AllGather", mybir.AluOpType.bypass, replica_groups=groups,
                ins=[tbl2c[:]], outs=[tbl2[:]],
            )

            for j in range(TPC_used):
                agg2 = midp.tile([P, 1], f32, tag="agg2")
                gather_reduce(
                    g2bp, j, tbl2, 64, f32, agg2[:],
                    lambda gb, sc: gb[:, 0:sc * 64].rearrange(
                        "p (s f) -> p f s", f=64)[:, 0:1, :])
                ot = midp.tile([P, 1], f32, tag="ot")
                nc.scalar.activation(out=ot[:], in_=agg2[:],
                                     func=mybir.ActivationFunctionType.Relu,
                                     scale=nd_sb[:, j:j + 1],
                                     bias=(b2_sb[:, 0:1] if b2_any else 0.0))
                nc.sync.dma_start(out=outp[j * P:(j + 1) * P, :], in_=ot[:])

    nc.compile()
    return nc


def _in_maps(pp):
    maps = []
    for c in range(C):
        m = dict(
            xT_hi=np.ascontiguousarray(pp["xT_hi"][c]),
            xT_lo=np.ascontiguousarray(pp["xT_lo"][c]),
            Whi=np.ascontiguousarray(pp["Whi"]),
            Wlo=np.ascontiguousarray(pp["Wlo"]),
            idx=np.ascontiguousarray(pp["idx16"][c]),
            ns=np.ascontiguousarray(pp["ns"][c]),
            scomb=np.ascontiguousarray(pp["scomb"][c]),
            nd=np.ascontiguousarray(pp["nd"][c]),
            w2rep=pp["w2rep"],
        )
        if pp["b1_any"]:
            m["b1rep"] = pp["b1rep"]
        if float(pp["b2"]) != 0.0:
            m["b2rep"] = np.full((P, 1), pp["b2"], np.float32)
        maps.append(m)
    return maps


_CACHE = {}


def kernel(x, src, dst, W1, b1, W2, b2, n_nodes):
    """Full inputs in, full [n_nodes, 1] float32 output out."""
    from concourse.bass_utils import run_bass_kernel_spmd

    pp = _preprocess(x, src, dst, W1, b1, W2, b2, n_nodes)
    key = (pp["N"], pp["TPC"], pp["TOTC"], tuple(pp["S_lo"]), tuple(pp["S_hi"]),
           pp["b1_any"], pp["b2"])
    if key not in _CACHE:
        _CACHE.clear()
        _CACHE[key] = _build(pp)
    nc = _CACHE[key]

    maps = _in_maps(pp)
    res = run_bass_kernel_spmd(nc, maps, core_ids=list(range(C)))

    N = pp["N"]
    out = np.zeros((N, 1), np.float32)
    loc = pp["pos_of"].astype(np.int64) * P + pp["part_of"]
    for c in range(C):
        sel = np.flatnonzero(pp["core_of"] == c)
        out[sel, 0] = res.results[c]["out"][loc[sel], 0]
    return out

